# revision 17
# baseline (speedup 1.0000x reference)
"""BitNet dense layer on 8 Trainium2 NeuronCores.

reference math:
    row_scale = clip(mean(|W|, axis=1), 1e-8)        # [out]
    out = (x @ sign(W).T) * row_scale * scale_param  # [B,S,out]

Strategy (data-parallel over the 8192 tokens, 1024 tokens/core):
  * The contraction dim K=4096 is split: K8=3840 columns run in fp8e4
    (E4M3) with perf_mode=DoubleRow — 2 fp8 weights per PE cell, 2
    MACs/cell/cycle, 2x the bf16 matmul rate (measured 1.92x) — and
    KB=256 columns run in bf16. Both parts accumulate into the SAME
    PSUM tile in one fused pass, so the PE stream never breaks.
  * sign(W) is exactly +-1 in fp8; only x pays quantization error.
    Host-side error feedback: the fp8 quantization residual eps is
    projected onto the bf16 columns (least squares via the sign
    matrix Gram) and added to the bf16 inputs, cancelling most of the
    fp8 error. Measured end-to-end max-rel error 1.80e-2 (< 2e-2),
    deterministic — HW matches the numpy model to 7 digits.
  * Per-row scale comb = row_scale * scale_param applied on host
    (rank-1 postscale), keeping the device kernel a pure matmul.

Device kernel (per core): out[1024, 4096] f32 = x8T.T @ w8T + xbT.T @ wbT
  - x tiles resident in SBUF (4.7 MB), w tiles double-buffered per
    n-slice of 512 outputs; psum [128, 512] accumulates 15 DoubleRow
    + 2 bf16 matmuls, evicted via DVE copy, DMA'd out.
"""

import numpy as np
import ml_dtypes

B, S, D_IN, D_OUT = 4, 2048, 4096, 4096
N_CORES = 8
M_TOT = B * S
M_LOC = M_TOT // N_CORES
K8 = 3840  # fp8 portion of K (multiple of 256)
KB = D_IN - K8  # bf16 portion (multiple of 256)
KS8 = K8 // 128  # fp8 k-subtiles (pairs of 2 -> DoubleRow)
KSB = KB // 128  # bf16 k-subtiles
N_TILE = 512
M_TILE = 128
N_TILES = D_OUT // N_TILE
M_TILES = M_LOC // M_TILE

_prog = None
last_results = None  # BassKernelResults of the most recent run (for test harness)
TRACE = False  # set True by the dev test harness (needs NTFF shims) to profile


def _build_program():
    import concourse.tile as tile
    from concourse import bacc, mybir

    nc = bacc.Bacc(
        "TRN2", target_bir_lowering=False, debug=False, num_devices=N_CORES
    )
    x8T = nc.dram_tensor(
        "x8T", [K8, M_LOC], mybir.dt.float8e4, kind="ExternalInput"
    ).ap()
    xbT = nc.dram_tensor(
        "xbT", [KB, M_LOC], mybir.dt.bfloat16, kind="ExternalInput"
    ).ap()
    # w tensors are host-blocked per n-tile ([N_TILES, K, N_TILE]) so
    # each tile's DMA reads one fully contiguous DRAM block.
    w8T = nc.dram_tensor(
        "w8T", [N_TILES, K8, N_TILE], mybir.dt.float8e4, kind="ExternalInput"
    ).ap()
    wbT = nc.dram_tensor(
        "wbT", [N_TILES, KB, N_TILE], mybir.dt.bfloat16, kind="ExternalInput"
    ).ap()
    out = nc.dram_tensor(
        "out", [M_LOC, D_OUT], mybir.dt.float32, kind="ExternalOutput"
    ).ap()

    x8T3 = x8T.rearrange("(po pi) f -> pi po f", pi=128)  # [128, KS8, M_LOC]
    xbT3 = xbT.rearrange("(po pi) f -> pi po f", pi=128)  # [128, KSB, M_LOC]
    w8T3 = w8T.rearrange("n (po pi) f -> pi n po f", pi=128)  # [128, NT, KS8, NTILE]
    wbT3 = wbT.rearrange("n (po pi) f -> pi n po f", pi=128)  # [128, NT, KSB, NTILE]

    DR = mybir.MatmulPerfMode.DoubleRow

    with tile.TileContext(nc) as tc:
        with (
            tc.tile_pool(name="warm", bufs=1) as warm,
            tc.tile_pool(name="warm_psum", bufs=1, space="PSUM") as warm_psum,
            tc.tile_pool(name="xpool", bufs=1) as xpool,
            tc.tile_pool(name="wpool", bufs=2) as wpool,
            tc.tile_pool(name="evict", bufs=4) as evict,
            tc.tile_pool(name="psum", bufs=4, space="PSUM") as psum_pool,
        ):
            # PE warmup: dummy matmuls run while the first real tiles DMA
            # in, releasing the HAM clock gate (1.2 -> 2.4 GHz takes
            # ~3.4us of PE activity) so the real stream starts fast.
            wa = warm.tile([128, 128], mybir.dt.bfloat16)
            wb = warm.tile([128, 512], mybir.dt.bfloat16)
            nc.vector.memset(wa[:], 0.0)
            nc.vector.memset(wb[:], 0.0)
            ps_w = warm_psum.tile([128, 512], mybir.dt.float32)
            for i in range(8):
                nc.tensor.matmul(
                    ps_w[:], wa[:], wb[:], start=(i == 0), stop=(i == 7)
                )

            # Inputs are DMA'd in k-subtile pieces: 1KB inner runs, and
            # the first matmuls only wait on the first piece thanks to
            # Tile's subtile dependency tracking.
            PIECE = 6

            def load_w(n):
                w8_t = wpool.tile(
                    [128, KS8, N_TILE], mybir.dt.float8e4, tag="w8"
                )
                for p in range(0, KS8, PIECE):
                    q = min(KS8, p + PIECE)
                    nc.sync.dma_start(
                        out=w8_t[:, p:q, :], in_=w8T3[:, n, p:q, :]
                    )
                wb_t = wpool.tile(
                    [128, KSB, N_TILE], mybir.dt.bfloat16, tag="wb"
                )
                nc.sync.dma_start(out=wb_t[:], in_=wbT3[:, n, :, :])
                return w8_t, wb_t

            # First n-tile's weight pieces are interleaved with the
            # resident x pieces so the first psum group's operands (w
            # piece0 + x piece0, ~1.1 MB) land first and the matmul
            # stream starts as early as possible.
            w8_0 = wpool.tile([128, KS8, N_TILE], mybir.dt.float8e4, tag="w8")
            x8_sb = xpool.tile([128, KS8, M_LOC], mybir.dt.float8e4, tag="x8")
            for p in range(0, KS8, PIECE):
                q = min(KS8, p + PIECE)
                nc.sync.dma_start(
                    out=w8_0[:, p:q, :], in_=w8T3[:, 0, p:q, :]
                )
                nc.sync.dma_start(out=x8_sb[:, p:q, :], in_=x8T3[:, p:q, :])
            wb_0 = wpool.tile([128, KSB, N_TILE], mybir.dt.bfloat16, tag="wb")
            nc.sync.dma_start(out=wb_0[:], in_=wbT3[:, 0, :, :])
            xb_sb = xpool.tile([128, KSB, M_LOC], mybir.dt.bfloat16, tag="xb")
            nc.sync.dma_start(out=xb_sb[:], in_=xbT3[:, :, :])
            w_next = (w8_0, wb_0)

            for n in range(N_TILES):
                n_sl = slice(n * N_TILE, (n + 1) * N_TILE)
                w8_t, wb_t = w_next
                if n + 1 < N_TILES:
                    w_next = load_w(n + 1)
                for m in range(M_TILES):
                    m_sl = slice(m * M_TILE, (m + 1) * M_TILE)
                    ps = psum_pool.tile([128, N_TILE], mybir.dt.float32)
                    for s in range(0, KS8, 2):
                        nc.tensor.matmul(
                            ps[:],
                            x8_sb[:, s : s + 2, m_sl],
                            w8_t[:, s : s + 2, :],
                            start=(s == 0),
                            stop=False,
                            perf_mode=DR,
                        )
                    for s in range(KSB):
                        nc.tensor.matmul(
                            ps[:],
                            xb_sb[:, s, m_sl],
                            wb_t[:, s, :],
                            start=False,
                            stop=(s == KSB - 1),
                        )
                    ev = evict.tile([128, N_TILE], mybir.dt.float32)
                    nc.vector.tensor_copy(out=ev[:], in_=ps[:])
                    nc.sync.dma_start(out=out[m_sl, n_sl], in_=ev[:])
    nc.compile()
    return nc


def kernel(input, weight, scale_param):
    global _prog, last_results
    from concourse.bass_utils import run_bass_kernel_spmd

    x = np.asarray(input, dtype=np.float32).reshape(M_TOT, D_IN)
    W = np.asarray(weight, dtype=np.float32)
    sp = np.asarray(scale_param, dtype=np.float32)

    comb = np.clip(np.abs(W).mean(axis=1, dtype=np.float32), 1e-8, None) * sp
    ST = np.sign(W).T.astype(np.float32)  # [D_IN, D_OUT]
    SF = ST[:K8]  # [K8, D_OUT]
    SB = ST[K8:]  # [KB, D_OUT]

    x8 = x[:, :K8].astype(ml_dtypes.float8_e4m3fn)  # [M, K8]
    # Error feedback: project the fp8 residual onto the bf16 columns
    # (least squares through the sign matrix) and fold into xb.
    eps = x[:, :K8] - x8.astype(np.float32)  # [M, K8]
    Mm = SF @ SB.T  # [K8, KB]
    G = (SB @ SB.T).astype(np.float64)  # [KB, KB]
    Ginv = np.linalg.inv(G).astype(np.float32)
    delta = (eps @ Mm) @ Ginv  # [M, KB]
    xb = (x[:, K8:] + delta).astype(ml_dtypes.bfloat16)  # [M, KB]

    x8T = np.ascontiguousarray(x8.T)
    xbT = np.ascontiguousarray(xb.T)
    # blocked per n-tile: [N_TILES, K, N_TILE], each tile contiguous
    w8T = np.ascontiguousarray(
        SF.astype(ml_dtypes.float8_e4m3fn)
        .reshape(K8, N_TILES, N_TILE)
        .transpose(1, 0, 2)
    )
    wbT = np.ascontiguousarray(
        SB.astype(ml_dtypes.bfloat16)
        .reshape(KB, N_TILES, N_TILE)
        .transpose(1, 0, 2)
    )

    if _prog is None:
        _prog = _build_program()

    in_maps = [
        {
            "x8T": np.ascontiguousarray(x8T[:, c * M_LOC : (c + 1) * M_LOC]),
            "xbT": np.ascontiguousarray(xbT[:, c * M_LOC : (c + 1) * M_LOC]),
            "w8T": w8T,
            "wbT": wbT,
        }
        for c in range(N_CORES)
    ]
    last_results = run_bass_kernel_spmd(
        _prog, in_maps, list(range(N_CORES)), trace=TRACE
    )
    out = np.concatenate(
        [last_results.results[c]["out"] for c in range(N_CORES)], axis=0
    )
    out *= comb[None, :]
    return np.nan_to_num(
        out.reshape(B, S, D_OUT), nan=0.0, posinf=1e6, neginf=-1e6
    )


# revision 18
# speedup vs baseline: 1.0736x; 1.0736x over previous
"""BitNet dense layer on 8 Trainium2 NeuronCores.

reference math:
    row_scale = clip(mean(|W|, axis=1), 1e-8)        # [out]
    out = (x @ sign(W).T) * row_scale * scale_param  # [B,S,out]

Strategy (data-parallel over the 8192 tokens, 1024 tokens/core):
  * The whole matmul runs in fp8e4 (E4M3) with perf_mode=DoubleRow:
    2 fp8 weights per PE cell, 2 MACs/cell/cycle — 2x the bf16 matmul
    rate (measured 1.92x end-to-end vs the 463us bf16 baseline).
  * sign(W) is exactly +-1 in fp8; only x pays quantization error.
    Pure e4m3 x measures 2.12e-2 max-rel error on these inputs — just
    over the 2e-2 gate — so the last 256 contraction columns are used
    as an error-feedback channel: the fp8 quantization residual of the
    first 3840 columns is least-squares-projected onto the sign
    columns of the last 256 (via the Gram matrix) and added to those x
    values before quantization. Measured end-to-end error: 1.878e-2,
    deterministic — HW matches this numpy model to 7 digits.
  * Per-row scale comb = row_scale * scale_param applied on host
    (rank-1 postscale), keeping the device kernel a pure matmul.

Device kernel (per core): out[1024, 4096] f32 = x8T.T @ w8T, all fp8:
  - x resident in SBUF (4 MB), w double-buffered per n-slice of 512
    outputs (host-blocked so each tile is one contiguous DRAM read);
    inputs DMA'd in k-subtile pieces so the first matmuls only wait on
    the first piece; psum [128, 512] accumulates 16 DoubleRow matmuls,
    evicted via DVE copy, DMA'd out.
"""

import numpy as np
import ml_dtypes

B, S, D_IN, D_OUT = 4, 2048, 4096, 4096
N_CORES = 8
M_TOT = B * S
M_LOC = M_TOT // N_CORES
K8 = 3840  # plain-quantized columns; the last KB carry error feedback
KB = D_IN - K8
KS = D_IN // 128  # k-subtiles (pairs of 2 -> DoubleRow)
N_TILE = 512
M_TILE = 128
N_TILES = D_OUT // N_TILE
M_TILES = M_LOC // M_TILE

_prog = None
last_results = None  # BassKernelResults of the most recent run (for test harness)
TRACE = False  # set True by the dev test harness (needs NTFF shims) to profile


def _build_program():
    import concourse.tile as tile
    from concourse import bacc, mybir

    nc = bacc.Bacc(
        "TRN2", target_bir_lowering=False, debug=False, num_devices=N_CORES
    )
    x8T = nc.dram_tensor(
        "x8T", [D_IN, M_LOC], mybir.dt.float8e4, kind="ExternalInput"
    ).ap()
    # host-blocked per n-tile so each tile's DMA is one contiguous read
    w8T = nc.dram_tensor(
        "w8T", [N_TILES, D_IN, N_TILE], mybir.dt.float8e4, kind="ExternalInput"
    ).ap()
    out = nc.dram_tensor(
        "out", [M_LOC, D_OUT], mybir.dt.float32, kind="ExternalOutput"
    ).ap()

    x8T3 = x8T.rearrange("(po pi) f -> pi po f", pi=128)  # [128, KS, M_LOC]
    w8T3 = w8T.rearrange("n (po pi) f -> pi n po f", pi=128)  # [128, NT, KS, NTILE]

    DR = mybir.MatmulPerfMode.DoubleRow

    with tile.TileContext(nc) as tc:
        with (
            tc.tile_pool(name="warm", bufs=1) as warm,
            tc.tile_pool(name="warm_psum", bufs=1, space="PSUM") as warm_psum,
            tc.tile_pool(name="xpool", bufs=1) as xpool,
            tc.tile_pool(name="wpool", bufs=2) as wpool,
            tc.tile_pool(name="evict", bufs=4) as evict,
            tc.tile_pool(name="psum", bufs=4, space="PSUM") as psum_pool,
        ):
            # PE warmup: dummy matmuls run while the first real tiles DMA
            # in, releasing the HAM clock gate (1.2 -> 2.4 GHz takes
            # ~3.4us of PE activity) so the real stream starts fast.
            wa = warm.tile([128, 128], mybir.dt.bfloat16)
            wb = warm.tile([128, 512], mybir.dt.bfloat16)
            nc.vector.memset(wa[:], 0.0)
            nc.vector.memset(wb[:], 0.0)
            ps_w = warm_psum.tile([128, 512], mybir.dt.float32)
            for i in range(8):
                nc.tensor.matmul(
                    ps_w[:], wa[:], wb[:], start=(i == 0), stop=(i == 7)
                )

            # Inputs are DMA'd in k-subtile pieces: the first matmuls
            # only wait on the first piece (Tile subtile deps).
            PIECE = 6

            def load_w(n):
                w8_t = wpool.tile(
                    [128, KS, N_TILE], mybir.dt.float8e4, tag="w8"
                )
                for p in range(0, KS, PIECE):
                    q = min(KS, p + PIECE)
                    nc.sync.dma_start(
                        out=w8_t[:, p:q, :], in_=w8T3[:, n, p:q, :]
                    )
                return w8_t

            # First n-tile's weight pieces are interleaved with the
            # resident x pieces so the first psum group's operands land
            # first and the matmul stream starts as early as possible.
            w8_0 = wpool.tile([128, KS, N_TILE], mybir.dt.float8e4, tag="w8")
            x8_sb = xpool.tile([128, KS, M_LOC], mybir.dt.float8e4, tag="x8")
            for p in range(0, KS, PIECE):
                q = min(KS, p + PIECE)
                nc.sync.dma_start(out=w8_0[:, p:q, :], in_=w8T3[:, 0, p:q, :])
                nc.sync.dma_start(out=x8_sb[:, p:q, :], in_=x8T3[:, p:q, :])
            w_next = w8_0

            for n in range(N_TILES):
                n_sl = slice(n * N_TILE, (n + 1) * N_TILE)
                w8_t = w_next
                if n + 1 < N_TILES:
                    w_next = load_w(n + 1)
                for m in range(M_TILES):
                    m_sl = slice(m * M_TILE, (m + 1) * M_TILE)
                    ps = psum_pool.tile([128, N_TILE], mybir.dt.float32)
                    for s in range(0, KS, 2):
                        nc.tensor.matmul(
                            ps[:],
                            x8_sb[:, s : s + 2, m_sl],
                            w8_t[:, s : s + 2, :],
                            start=(s == 0),
                            stop=(s == KS - 2),
                            perf_mode=DR,
                        )
                    ev = evict.tile([128, N_TILE], mybir.dt.float32)
                    nc.vector.tensor_copy(out=ev[:], in_=ps[:])
                    nc.sync.dma_start(out=out[m_sl, n_sl], in_=ev[:])
    nc.compile()
    return nc


def kernel(input, weight, scale_param):
    global _prog, last_results
    from concourse.bass_utils import run_bass_kernel_spmd

    x = np.asarray(input, dtype=np.float32).reshape(M_TOT, D_IN)
    W = np.asarray(weight, dtype=np.float32)
    sp = np.asarray(scale_param, dtype=np.float32)

    comb = np.clip(np.abs(W).mean(axis=1, dtype=np.float32), 1e-8, None) * sp
    ST = np.sign(W).T.astype(np.float32)  # [D_IN, D_OUT]
    SF = ST[:K8]
    SB = ST[K8:]

    x8 = x[:, :K8].astype(ml_dtypes.float8_e4m3fn)  # [M, K8]
    # Error feedback: project the fp8 residual of the first K8 columns
    # onto the last KB sign columns (least squares via the Gram matrix)
    # and fold into those x values before quantizing them too.
    eps = x[:, :K8] - x8.astype(np.float32)
    Mm = SF @ SB.T  # [K8, KB]
    G = (SB @ SB.T).astype(np.float64)  # [KB, KB]
    Ginv = np.linalg.inv(G).astype(np.float32)
    delta = (eps @ Mm) @ Ginv  # [M, KB]
    xc = (x[:, K8:] + delta).astype(ml_dtypes.float8_e4m3fn)

    x8full = np.concatenate([x8, xc], axis=1)  # [M, D_IN] fp8
    x8T = np.ascontiguousarray(x8full.T)
    w8T = np.ascontiguousarray(
        ST.astype(ml_dtypes.float8_e4m3fn)
        .reshape(D_IN, N_TILES, N_TILE)
        .transpose(1, 0, 2)
    )

    if _prog is None:
        _prog = _build_program()

    in_maps = [
        {
            "x8T": np.ascontiguousarray(x8T[:, c * M_LOC : (c + 1) * M_LOC]),
            "w8T": w8T,
        }
        for c in range(N_CORES)
    ]
    last_results = run_bass_kernel_spmd(
        _prog, in_maps, list(range(N_CORES)), trace=TRACE
    )
    out = np.concatenate(
        [last_results.results[c]["out"] for c in range(N_CORES)], axis=0
    )
    out *= comb[None, :]
    return np.nan_to_num(
        out.reshape(B, S, D_OUT), nan=0.0, posinf=1e6, neginf=-1e6
    )
